# revision 26
# baseline (speedup 1.0000x reference)
"""Trainium2 Bass kernel for nn_Encoder_83846351553267.

Net: conv2d(7x7,s2)x2 stem -> 3 layers of [conv1d(7,s2,dense) + 2 Mamba2 blocks].
(dm, L) per layer: (256,256), (512,128), (1024,64); batch 4; out (4,1024,64) f32.

Strategy:
  - Data-parallel over batch: core b computes batch b fully. No collectives.
  - Selective scan in SSD quadratic form (one chunk, h0=0, decays<=1):
      y[:,t] per head = sum_s  G[s,t] * exp(s_t - s_s) * dt_s * [s<=t] * x[:,s]
    with G = B^T C shared across heads (ngroups=1). All heavy ops are PE matmuls.
  - Channels-major layout [channels(partitions), positions(free)] end to end.
  - fp16 matmul operands; fp32 for dt/softplus/cumsum/exp/rmsnorm.
"""
import numpy as np
from contextlib import ExitStack

import concourse.bass as bass
import concourse.bacc as bacc
import concourse.tile as tile
import concourse.mybir as mybir
from concourse import masks
from concourse.bass_utils import run_bass_kernel_spmd

F16 = mybir.dt.float16
F32 = mybir.dt.float32
F32R = mybir.dt.float32r
OP = mybir.AluOpType
AF = mybir.ActivationFunctionType

D_STATE = 128
HEADDIM = 64
NEG = -30000.0
LAYERS = [(128, 256, 256), (256, 512, 128), (512, 1024, 64)]  # (C_in, dm, L)
N_CORES = 4


# =================================================================== host prep
def _prep_weights(params):
    w = {}
    dvals = {}
    w1 = np.asarray(params['conv1'], np.float32)   # (2,1,7,7)
    l1 = np.zeros((7, 69, 64), np.float32)
    for oc in range(2):
        for gy in range(32):
            for ky in range(7):
                for kx in range(7):
                    l1[kx, 2 * gy + ky, oc * 32 + gy] = w1[oc, 0, ky, kx]
    w['conv1_lhsT'] = l1.astype(np.float16)
    w2 = np.asarray(params['conv2'], np.float32)   # (4,2,7,7)
    l2 = np.zeros((7, 2, 37, 64), np.float32)
    for oc in range(4):
        for gy in range(16):
            for ic in range(2):
                for ky in range(7):
                    for kx in range(7):
                        l2[kx, ic, 2 * gy + ky, oc * 16 + gy] = w2[oc, ic, ky, kx]
    w['conv2_lhsT'] = l2.astype(np.float16)

    for li, lp in enumerate(params['layers']):
        cw = np.asarray(lp['conv_w'], np.float32)          # (2C, C, 7)
        w[f'l{li}_convw'] = np.ascontiguousarray(cw.transpose(2, 1, 0)).astype(np.float16)
        for bi, bp in enumerate(lp['blocks']):
            P = np.asarray(bp['in_proj'], np.float32)      # (feat, dm)
            dm = P.shape[1]
            di = 2 * dm
            H = di // HEADDIM
            pre = f'l{li}b{bi}_'
            w[pre + 'wz'] = np.ascontiguousarray(P[:di].T).astype(np.float16)
            w[pre + 'wx'] = np.ascontiguousarray(P[di:2 * di].T).astype(np.float16)
            w[pre + 'wb'] = np.ascontiguousarray(P[2 * di:2 * di + D_STATE].T).astype(np.float16)
            w[pre + 'wc'] = np.ascontiguousarray(
                P[2 * di + D_STATE:2 * di + 2 * D_STATE].T).astype(np.float16)
            w[pre + 'wdt'] = np.ascontiguousarray(P[-H:].T).astype(np.float16)
            w[pre + 'w4'] = np.asarray(bp['conv_w'], np.float32)[:, 0, :].copy()
            w[pre + 'cb'] = np.asarray(bp['conv_b'], np.float32)[:, None].copy()
            w[pre + 'dtb'] = np.asarray(bp['dt_bias'], np.float32)[:, None].copy()
            w[pre + 'A'] = (-np.exp(np.asarray(bp['A_log'], np.float32)))[:, None].copy()
            wo = np.asarray(bp['out_proj'], np.float32) * \
                np.asarray(bp['norm_w'], np.float32)[None, :]
            w[pre + 'wo'] = np.ascontiguousarray(wo.T).astype(np.float16)
            dvals[pre] = [float(x) for x in np.asarray(bp['D'], np.float32)]
    return w, dvals


def _ceil(a, b):
    return (a + b - 1) // b


# ================================================================ bass program
def build_bass(wshapes, dvals, debug=False):
    nc = bacc.Bacc("TRN2", target_bir_lowering=False)

    dram = {}
    for name, (shape, dt_np) in wshapes.items():
        dt_b = F16 if dt_np == np.float16 else F32
        dram[name] = nc.dram_tensor(name, list(shape), dt_b, kind="ExternalInput")
    tokens_d = nc.dram_tensor("tokens", [134, 2054], F16, kind="ExternalInput")
    out_d = nc.dram_tensor("out", [1024, 64], F32, kind="ExternalOutput")

    cneg_d = {}
    for (_, _, L) in LAYERS:
        cneg_d[L] = nc.inline_tensor(
            np.tril(np.full((L, L), NEG, np.float32), -1), name=f"cneg{L}")

    dbg = {}
    if debug:
        dbg['stem2'] = nc.dram_tensor("dbg_stem2", [128, 512], F32, kind="ExternalOutput")
        dbg['l0conv'] = nc.dram_tensor("dbg_l0conv", [256, 256], F32, kind="ExternalOutput")
        dbg['l0b0'] = nc.dram_tensor("dbg_l0b0", [256, 256], F32, kind="ExternalOutput")

    with tile.TileContext(nc) as tc, ExitStack() as ctx:
        def pool(name, bufs):
            return ctx.enter_context(tc.tile_pool(name=name, bufs=bufs))
        const_p = pool("const", 1)
        stem_p = pool("stem", 2)
        u_p = pool("u", 18)
        xbc_p = pool("xbc", 18)
        xc_p = pool("xc", 18)
        xt_p = pool("xt", 2)
        zs_p = pool("zs", 16)
        msk_p = pool("msk", 3)
        sml_p = pool("sml", 2)
        wbig_p = pool("wbig", 9)
        wsm_p = pool("wsm", 9)
        wo_p = pool("wo", 17)
        cw_p = pool("cw", 28)
        ws_p = pool("wtiny", 19)
        psA = ctx.enter_context(tc.tile_pool(name="psA", bufs=2, space="PSUM"))
        psG = ctx.enter_context(tc.tile_pool(name="psG", bufs=2, space="PSUM"))
        psR = ctx.enter_context(tc.tile_pool(name="psR", bufs=2, space="PSUM"))
        psY = ctx.enter_context(tc.tile_pool(name="psY", bufs=2, space="PSUM"))

        ident = const_p.tile([128, 128], F32)
        masks.make_identity(nc, ident[:])
        ident16 = const_p.tile([128, 128], F16)
        masks.make_identity(nc, ident16[:])
        ones_row = const_p.tile([1, 128], F32)
        nc.gpsimd.memset(ones_row[:], 1.0)
        onesK = const_p.tile([128, 1], F16)
        nc.gpsimd.memset(onesK[:], 1.0)
        zer_g = const_p.tile([32, 256], F32)
        nc.gpsimd.memset(zer_g[:], 0.0)

        cneg_sb = {}
        for (_, _, L) in LAYERS:
            tiles = []
            for scn in range(_ceil(L, 128)):
                p = min(128, L - scn * 128)
                t = const_p.tile([p, L], F32, tag=f"cneg{L}_{scn}")
                nc.sync.dma_start(t[:], cneg_d[L][scn * 128:scn * 128 + p, :])
                tiles.append(t)
            cneg_sb[L] = tiles

        # ------------------------------------------------------------- stem
        # conv2d as K=vertical-band matmuls, contracting kx via 7 accumulating
        # MMs whose rhs are plain strided slices — no im2col gathers at all.
        tokA = stem_p.tile([128, 2054], F16, tag="tokA", bufs=1)
        nc.sync.dma_start(tokA[:], tokens_d[0:128, :])
        tokB = stem_p.tile([70, 2054], F16, tag="tokB", bufs=1)
        nc.sync.dma_start(tokB[:], tokens_d[64:134, :])

        w1t = []
        for kx in range(7):
            t = stem_p.tile([69, 64], F16, tag="w1t", bufs=7, name=f"w1t{kx}")
            nc.sync.dma_start(t[:], dram['conv1_lhsT'][kx])
            w1t.append(t)
        w2t = {}
        for kx in range(7):
            for ic in range(2):
                t = stem_p.tile([37, 64], F16, tag="w2t", bufs=14, name=f"w2t{kx}_{ic}")
                nc.sync.dma_start(t[:], dram['conv2_lhsT'][kx, ic])
                w2t[(kx, ic)] = t

        stem1d = nc.dram_tensor("stem1d", [2, 70, 1030], F16)
        zfill_d = nc.inline_tensor(np.zeros((2, 70, 1030), np.float16), name="zfill")
        nc.sync.dma_start(stem1d[:], zfill_d[:])

        for g in range(2):                       # conv1, Gy=32
            rows = tokA[0:69, :] if g == 0 else tokB[0:69, :]
            for nt in range(2):
                ps = psA.tile([64, 512], F32, tag="mm")
                for kx in range(7):
                    nc.tensor.matmul(ps[:], w1t[kx][:],
                                     rows[:, kx + nt * 1024:kx + nt * 1024 + 1024:2],
                                     start=(kx == 0), stop=(kx == 6))
                stg = stem_p.tile([64, 512], F16, tag="st1", bufs=4, name="stg1")
                nc.vector.tensor_copy(stg[:], ps[:])
                for oc in range(2):
                    nc.sync.dma_start(
                        stem1d[oc, 3 + g * 32:3 + g * 32 + 32,
                               3 + nt * 512:3 + (nt + 1) * 512],
                        stg[oc * 32:(oc + 1) * 32, :])

        tc.strict_bb_all_engine_barrier()

        s1sb = {}
        for ic in range(2):
            for g in range(2):
                t = stem_p.tile([37, 1030], F16, tag="s1sb", bufs=4, name=f"s1sb{ic}{g}")
                nc.sync.dma_start(t[:], stem1d[ic, 32 * g:32 * g + 37, :])
                s1sb[(ic, g)] = t

        stem2 = u_p.tile([128, 518], F16, tag="u0", bufs=1)
        nc.gpsimd.memset(stem2[:, 0:3], 0.0)
        nc.gpsimd.memset(stem2[:, 515:518], 0.0)
        for g in range(2):                       # conv2, Gy=16
            ps = psA.tile([64, 512], F32, tag="mm")
            first = True
            for kx in range(7):
                for ic in range(2):
                    nc.tensor.matmul(ps[:], w2t[(kx, ic)][:],
                                     s1sb[(ic, g)][:, kx:kx + 1024:2],
                                     start=first, stop=(kx == 6 and ic == 1))
                    first = False
            stg = stem_p.tile([64, 512], F16, tag="st2", bufs=2, name="stg2")
            nc.vector.tensor_copy(stg[:], ps[:])
            for oc in range(4):
                nc.sync.dma_start(
                    stem2[oc * 32 + g * 16:oc * 32 + g * 16 + 16, 3:515],
                    stg[oc * 16:(oc + 1) * 16, :])

        tc.strict_bb_all_engine_barrier()

        if debug:
            f = stem_p.tile([128, 512], F32, tag="dbg")
            nc.vector.tensor_copy(f[:], stem2[:, 3:515])
            nc.sync.dma_start(dbg['stem2'][:], f[:])

        # ------------------------------------------------------------ layers
        u_tiles = [stem2]
        for li, (Cin, dm, L) in enumerate(LAYERS):
            nCin = Cin // 128
            nCo = 2 * Cin // 128
            Lin = 2 * L

            # ---- conv1d (dense, stride 2): two co-half passes ----
            new_u = []
            for half in range(2):
                m0 = half * (Cin // 128)        # m-chunks per half = nCo/2
                cw = {}
                for kx in range(7):
                    for k in range(nCin):
                        t = cw_p.tile([128, Cin], F16, tag="convw")
                        nc.sync.dma_start(
                            t[:], dram[f'l{li}_convw'][kx, k * 128:(k + 1) * 128,
                                                       half * Cin:(half + 1) * Cin])
                        cw[(kx, k)] = t
                for mloc in range(Cin // 128):
                    ps = psA.tile([128, L], F32, tag="mm")
                    first = True
                    for kx in range(7):
                        for k in range(nCin):
                            nc.tensor.matmul(
                                ps[:], cw[(kx, k)][:, mloc * 128:(mloc + 1) * 128],
                                u_tiles[k][:, kx:kx + 2 * L:2],
                                start=first, stop=(kx == 6 and k == nCin - 1))
                            first = False
                    t = u_p.tile([128, L + 6], F16, tag=f"u{li + 1}", bufs=[6, 10, 18][li])
                    nc.gpsimd.memset(t[:, 0:3], 0.0)
                    nc.gpsimd.memset(t[:, L + 3:L + 6], 0.0)
                    nc.vector.tensor_copy(t[:, 3:3 + L], ps[:])
                    new_u.append(t)
            u_tiles = new_u
            if debug and li == 0:
                for m in range(2):
                    f = stem_p.tile([128, 256], F32, tag="dbg2")
                    nc.vector.tensor_copy(f[:], u_tiles[m][:, 3:259])
                    nc.sync.dma_start(dbg['l0conv'][m * 128:(m + 1) * 128, :], f[:])

            # ---- mamba blocks ----
            d_inner = 2 * dm
            H = d_inner // HEADDIM
            nDi = d_inner // 128
            nDm = dm // 128
            nK = dm // 128
            conv_chunks = nDi + 2
            nSc = _ceil(L, 128)

            for bi in range(2):
                pre = f'l{li}b{bi}_'
                last_block = (li == 2 and bi == 1)
                rhs_aps = [u[:, 3:3 + L] for u in u_tiles]

                w4sb, cbsb = [], []
                for c in range(conv_chunks):
                    t4 = ws_p.tile([128, 4], F32, tag="w4")
                    nc.sync.dma_start(t4[:], dram[pre + 'w4'][c * 128:(c + 1) * 128, :])
                    w4sb.append(t4)
                    tb = ws_p.tile([128, 1], F32, tag="cb")
                    nc.sync.dma_start(tb[:], dram[pre + 'cb'][c * 128:(c + 1) * 128, :])
                    cbsb.append(tb)
                dtb = ws_p.tile([H, 1], F32, tag="dtb")
                nc.sync.dma_start(dtb[:], dram[pre + 'dtb'][:])
                Acol = ws_p.tile([H, 1], F32, tag="A")
                nc.sync.dma_start(Acol[:], dram[pre + 'A'][:])

                z_silu = [zs_p.tile([128, L], F16, tag=f"zs{li}", name=f"zsilu{c}", bufs=nDi) for c in range(nDi)]
                xbc_raw = [xbc_p.tile([128, L + 3], F16, tag=f"xbc{li}", name=f"xbcr{c}", bufs=conv_chunks)
                           for c in range(conv_chunks)]
                for t in xbc_raw:
                    nc.gpsimd.memset(t[:, 0:3], 0.0)
                dt_sb = sml_p.tile([H, L], F32, tag="dt")

                # z / x in two M-halves to halve live weight footprint
                hd = d_inner // 2
                for wname, writers_all in (
                    (pre + 'wz', [('z', c) for c in range(nDi)]),
                    (pre + 'wx', [('x', c) for c in range(nDi)]),
                ):
                    for half in range(2):
                        wt = []
                        for k in range(nK):
                            t = wbig_p.tile([128, hd], F16, tag="wbig")
                            nc.sync.dma_start(
                                t[:], dram[wname][k * 128:(k + 1) * 128,
                                                  half * hd:(half + 1) * hd])
                            wt.append(t)
                        for cloc in range(nDi // 2):
                            c = half * (nDi // 2) + cloc
                            kind = writers_all[c][0]
                            ps = psA.tile([128, L], F32, tag="mm")
                            for k in range(nK):
                                nc.tensor.matmul(ps[:], wt[k][:, cloc * 128:(cloc + 1) * 128],
                                                 rhs_aps[k], start=(k == 0),
                                                 stop=(k == nK - 1))
                            if kind == 'z':
                                sg = msk_p.tile([128, L], F16, tag="sg", name="sg")
                                nc.scalar.activation(sg[:], ps[:], AF.Sigmoid)
                                nc.vector.tensor_tensor(z_silu[c][:], sg[:], ps[:],
                                                        OP.mult)
                            else:
                                nc.vector.tensor_copy(xbc_raw[c][:, 3:3 + L], ps[:])

                # B, C, dt
                for wname, mw, writer in (
                    (pre + 'wb', D_STATE,
                     lambda ps: nc.vector.tensor_copy(xbc_raw[nDi][:, 3:3 + L], ps[:])),
                    (pre + 'wc', D_STATE,
                     lambda ps: nc.vector.tensor_copy(xbc_raw[nDi + 1][:, 3:3 + L], ps[:])),
                    (pre + 'wdt', H, 'softplus'),
                ):
                    wt = []
                    for k in range(nK):
                        t = wsm_p.tile([128, mw], F16, tag="wsm")
                        nc.sync.dma_start(t[:], dram[wname][k * 128:(k + 1) * 128, :])
                        wt.append(t)
                    ps = psA.tile([mw, L], F32, tag="mm")
                    for k in range(nK):
                        nc.tensor.matmul(ps[:], wt[k][:], rhs_aps[k],
                                         start=(k == 0), stop=(k == nK - 1))
                    if writer == 'softplus':
                        esp = sml_p.tile([H, L], F32, tag="esp")
                        nc.scalar.activation(esp[:], ps[:], AF.Exp, bias=dtb[:])
                        nc.vector.tensor_scalar(esp[:], esp[:], 1.0, None, op0=OP.add)
                        nc.scalar.activation(dt_sb[:], esp[:], AF.Ln)
                    else:
                        writer(ps)

                # ---- depthwise conv + silu ----
                xc = [xc_p.tile([128, L], F16, tag=f"xc{li}", name=f"xcc{c}", bufs=conv_chunks) for c in range(conv_chunks)]
                for c in range(conv_chunks):
                    acc = msk_p.tile([128, L], F32, tag="cacc")
                    nc.vector.tensor_scalar(acc[:], xbc_raw[c][:, 0:L], w4sb[c][:, 0:1],
                                            None, op0=OP.mult)
                    for k in range(1, 4):
                        nc.vector.scalar_tensor_tensor(acc[:], xbc_raw[c][:, k:k + L],
                                                       w4sb[c][:, k:k + 1], acc[:],
                                                       OP.mult, OP.add)
                    nc.vector.tensor_scalar(acc[:], acc[:], cbsb[c][:, 0:1], None,
                                            op0=OP.add)
                    sg2 = msk_p.tile([128, L], F32, tag="sg2", name="sg2")
                    nc.scalar.activation(sg2[:], acc[:], AF.Sigmoid)
                    nc.vector.tensor_tensor(xc[c][:], sg2[:], acc[:], OP.mult)
                xB, xC = xc[nDi], xc[nDi + 1]

                # ---- dt prep ----
                dtA = sml_p.tile([H, L], F32, tag="dtA")
                nc.vector.tensor_scalar(dtA[:], dt_sb[:], Acol[:, 0:1], None, op0=OP.mult)
                s_cum = sml_p.tile([H, L], F32, tag="scum")
                nc.vector.tensor_tensor_scan(s_cum[:], dtA[:], zer_g[0:H, 0:L], 0.0, OP.add, OP.add)

                sT_neg, dtT = [], []
                for sc in range(nSc):
                    p = min(128, L - sc * 128)
                    pst = psR.tile([p, H], F32, tag="r")
                    nc.tensor.transpose(pst[:], s_cum[:, sc * 128:sc * 128 + p], ident[0:H, 0:H])
                    t1 = sml_p.tile([p, H], F32, tag="sT")
                    nc.scalar.mul(t1[:], pst[:], -1.0)
                    sT_neg.append(t1)
                    pst2 = psR.tile([p, H], F32, tag="r")
                    nc.tensor.transpose(pst2[:], dt_sb[:, sc * 128:sc * 128 + p], ident[0:H, 0:H])
                    t2 = sml_p.tile([p, H], F32, tag="dtT")
                    nc.vector.tensor_copy(t2[:], pst2[:])
                    dtT.append(t2)

                # ---- G^T per s-chunk ----
                gt_ps = []
                for sc in range(nSc):
                    p = min(128, L - sc * 128)
                    ps = psG.tile([p, L], F32, tag="gt")
                    nc.tensor.matmul(ps[:], xB[:, sc * 128:sc * 128 + p], xC[:],
                                     start=True, stop=True)
                    gt_ps.append(ps)

                # ---- x^T ----
                xT_list = []
                for sc in range(nSc):
                    p = min(128, L - sc * 128)
                    xt = xt_p.tile([p, d_inner], F16, tag="xT")
                    if L >= 128:
                        for cc in range(nDi):
                            nc.sync.dma_start_transpose(
                                xt[:, cc * 128:(cc + 1) * 128],
                                xc[cc][:, sc * 128:(sc + 1) * 128])
                    else:
                        for cc in range(nDi):
                            pst = psR.tile([L, 128], F16, tag="r")
                            nc.tensor.transpose(pst[:], xc[cc][:], ident16[:])
                            nc.vector.tensor_copy(xt[:, cc * 128:(cc + 1) * 128], pst[:])
                    xT_list.append(xt)

                # ---- per-head SSD ----
                y_tiles = [zs_p.tile([128, L], F16, tag=f"yt{li}", name=f"ytl{c}", bufs=nDi) for c in range(nDi)]
                for h in range(H):
                    srow = sml_p.tile([1, L], F32, tag="srow", bufs=4)
                    nc.sync.dma_start(srow[:], s_cum[h:h + 1, :])
                    rps = psR.tile([128, L], F32, tag="r")
                    nc.tensor.matmul(rps[:], ones_row[:], srow[:],
                                     start=True, stop=True)
                    yps = psY.tile([HEADDIM, L], F32, tag="y")
                    for sc in range(nSc):
                        p = min(128, L - sc * 128)
                        tmp = msk_p.tile([p, L], F32, tag="tmp")
                        nc.vector.tensor_tensor(tmp[:], rps[0:p, :], cneg_sb[L][sc][:],
                                                OP.add)
                        E = msk_p.tile([p, L], F16, tag="E")
                        nc.scalar.activation(E[:], tmp[:], AF.Exp,
                                             bias=sT_neg[sc][:, h:h + 1])
                        Ab = msk_p.tile([p, L], F16, tag="Ab")
                        nc.vector.scalar_tensor_tensor(Ab[:], E[:], dtT[sc][:, h:h + 1],
                                                       gt_ps[sc][:], OP.mult, OP.mult)
                        nc.tensor.matmul(yps[:], xT_list[sc][:, h * 64:(h + 1) * 64],
                                         Ab[:], start=(sc == 0), stop=(sc == nSc - 1))
                    cc, off = h // 2, (h % 2) * 64
                    nc.vector.scalar_tensor_tensor(
                        y_tiles[cc][off:off + 64, :], xc[cc][off:off + 64, :],
                        dvals[pre][h], yps[:], OP.mult, OP.add)

                # ---- gating + rms-norm scale ----
                ssq = psY.tile([1, L], F32, tag="y")
                for c in range(nDi):
                    nc.vector.tensor_tensor(y_tiles[c][:], y_tiles[c][:], z_silu[c][:],
                                            OP.mult)
                    sq = msk_p.tile([128, L], F16, tag="sq")
                    nc.scalar.activation(sq[:], y_tiles[c][:], AF.Square)
                    nc.tensor.matmul(ssq[:], onesK[:], sq[:],
                                     start=(c == 0), stop=(c == nDi - 1))
                vrow = sml_p.tile([1, L], F32, tag="vrow")
                nc.scalar.activation(vrow[:], ssq[:], AF.Copy, bias=1e-5,
                                     scale=1.0 / d_inner)
                rrow = sml_p.tile([1, L], F32, tag="rrow")
                nc.vector.reciprocal(rrow[:], vrow[:])
                srow2 = sml_p.tile([1, L], F32, tag="srow2")
                nc.scalar.activation(srow2[:], rrow[:], AF.Sqrt)
                scb_ps = psR.tile([128, L], F32, tag="r")
                nc.tensor.matmul(scb_ps[:], ones_row[:], srow2[:],
                                 start=True, stop=True)
                scb = sml_p.tile([128, L], F32, tag="scb")
                nc.vector.tensor_copy(scb[:], scb_ps[:])

                # ---- out_proj (rms scale folded at evacuation) ----
                out_tiles = []
                for half in range(2):
                    hw = dm // 2
                    wt = []
                    for k in range(nDi):
                        t = wo_p.tile([128, hw], F16, tag="wo")
                        nc.sync.dma_start(t[:], dram[pre + 'wo'][k * 128:(k + 1) * 128,
                                                                 half * hw:(half + 1) * hw])
                        wt.append(t)
                    for mloc in range(nDm // 2):
                        ps = psA.tile([128, L], F32, tag="mm")
                        for k in range(nDi):
                            nc.tensor.matmul(ps[:], wt[k][:, mloc * 128:(mloc + 1) * 128],
                                             y_tiles[k][:], start=(k == 0),
                                             stop=(k == nDi - 1))
                        m_abs = half * (nDm // 2) + mloc
                        if last_block:
                            f = u_p.tile([128, L], F32, tag="uf", bufs=3)
                            nc.vector.tensor_tensor(f[:], ps[:], scb[:], OP.mult)
                            nc.sync.dma_start(out_d[m_abs * 128:(m_abs + 1) * 128, :], f[:])
                        else:
                            t = u_p.tile([128, L + 6], F16, tag=f"u{li + 1}", bufs=[6, 10, 18][li])
                            nc.gpsimd.memset(t[:, 0:3], 0.0)
                            nc.gpsimd.memset(t[:, L + 3:L + 6], 0.0)
                            nc.vector.tensor_tensor(t[:, 3:3 + L], ps[:], scb[:],
                                                    OP.mult)
                            out_tiles.append(t)
                if not last_block:
                    u_tiles = out_tiles
                if debug and li == 0 and bi == 0:
                    for m in range(2):
                        f = stem_p.tile([128, 256], F32, tag="dbg2")
                        nc.vector.tensor_copy(f[:], u_tiles[m][:, 3:259])
                        nc.sync.dma_start(dbg['l0b0'][m * 128:(m + 1) * 128, :], f[:])

    nc.compile()
    return nc


# ==================================================================== entry
_CACHE = {}


def kernel(**inputs):
    tokens = np.asarray(inputs['tokens'], np.float32)    # (4, 1, 128, 2048)
    params = inputs['params']
    w, dvals = _prep_weights(params)
    wshapes = {k: (v.shape, v.dtype.type) for k, v in w.items()}

    key = 'nc'
    if key not in _CACHE:
        _CACHE[key] = build_bass(wshapes, dvals, debug=False)
    nc = _CACHE[key]

    in_maps = []
    for b in range(N_CORES):
        m = dict(w)
        tp = np.zeros((134, 2054), np.float16)
        tp[3:131, 3:2051] = tokens[b, 0].astype(np.float16)
        m['tokens'] = tp
        in_maps.append(m)
    res = run_bass_kernel_spmd(nc, in_maps, core_ids=list(range(N_CORES)))
    return np.stack([r['out'] for r in res.results]).astype(np.float32)
